# revision 46
# baseline (speedup 1.0000x reference)
"""BitMambaBlock TRN2 kernel — 8-core SPMD via bass/Tile.

Sharding: d_inner split 8 ways (256 channels/core).
 - in_proj / conv / dt_proj / scan are fully local per d_inner shard.
 - x_proj contraction over d_inner -> per-core partial + in-NEFF AllReduce,
   one AllReduce per batch b so AR(b0) overlaps conv/x_proj(b1) and AR(b1)
   overlaps dt/scan(b0).
 - out stage: per-b AllToAll re-shards to 128-token waves (each core gets
   tokens [c*128:(c+1)*128] of batch b), with the per-token absmax row
   piggybacked on the payload so the output quantization scale needs no
   extra collective.  Wave-0 output compute hides under scan(b1)/a2a(b1).

Numerics:
 - BitNet weight ternarization + per-row scales precomputed on host (exact).
 - Activation int8 quantization on device; the int values live in bf16
   (ints <=127 exact) so both big matmuls are exact-integer bf16 PE matmuls.
 - SSM scan uses hw tensor_tensor_scan (fp32 internal recurrence); dA =
   exp(-A*dt) on ACT in bf16; dt stored bf16.
 - y = sum_n C_n*h_n accumulated on the PE via identity-matmul PSUM adds.
"""
import sys, os
for _p in ("/opt/trn_rl_repo", "/root/.axon_site/_ro/trn_rl_repo"):
    if os.path.isdir(_p) and _p not in sys.path:
        sys.path.insert(0, _p)

from contextlib import ExitStack
import numpy as np
import ml_dtypes

import concourse.bass as bass
import concourse.tile as tile
from concourse import bacc, mybir, bass_isa
from concourse.bass_utils import run_bass_kernel_spmd

F32 = mybir.dt.float32
BF16 = mybir.dt.bfloat16
AOP = mybir.AluOpType
AF = mybir.ActivationFunctionType

B, S, DM, DI, NST, DTR, DC = 2, 1024, 1024, 2048, 16, 64, 4
NCORES = 8
DL = DI // NCORES          # 256 d_inner channels per core
TOK = B * S                # 2048 tokens
TL = TOK // NCORES         # 256 tokens per core total (128 per wave/batch)
WL = S // NCORES           # 128 tokens per wave
RPC = 2 * 128 + 1          # a2a rows per chunk: 256 y2 rows + 1 absmax row
MAGIC = 12582912.0         # 1.5*2^23: (x+M)-M == round-to-nearest-even, |x|<2^22

U_BF16 = True              # u = dtx*B in bf16
H_BF16 = True              # scan output h in bf16
GP_FRAC_NUM, GP_FRAC_DEN = 0, 5   # fraction of u/w muls routed to gpsimd
STG1_AQB = "dve"    # engine for stage-1 round-to-int: pool | dve
STG1_CP = "act"     # engine for stage-1 aqT copies: act | dve
SHIP_MAX = "recv"   # out-scale path: piggyback (gpsimd partition ops) | recv
COLL = "real"       # real | fake (local copy, timing-only)
DA_BF16 = True      # dA in bf16 (16-bit scan operands)
DTXB_POOL = False   # dtx mult on gpsimd
STG1_T1 = "act"     # engine for in_proj f_in scale: act | dve

_CACHE = {}
import os as _os
KSTOP = _os.environ.get("KSTOP", "full")

# Steer Exp and Ln to the combined natural_log_exp table so the ACT engine
# never reloads its function table when the scheduler interleaves them.
# Table ids stay positional into act_info.json; only the chooser's view of
# which table "owns" Exp/Ln changes.
import concourse.bacc as _bacc_mod
from concourse.hw_specs import get_activation_tables as _gat_orig

def _gat_patched(arch):
    t = dict(_gat_orig(arch))
    exp, ln = AF.Exp, AF.Ln
    comb = t.get("natural_log_exp_and_others")
    if comb and exp in comb and ln in comb:
        if "exp_and_others" in t:
            t["exp_and_others"] = t["exp_and_others"] - {exp}
        if "natural_log" in t:
            t["natural_log"] = t["natural_log"] - {ln}
    return t

_bacc_mod.get_activation_tables = _gat_patched


def _bcast_rows(dram_ap, row0, nrows, col0, ncols, row_stride, parts=128):
    """[0-stride x parts] broadcast AP over dram_ap[row0:row0+nrows, col0:col0+ncols]."""
    a = dram_ap[row0:row0 + 1, col0:col0 + ncols]
    return bass.AP(tensor=a.tensor, offset=a.offset,
                   ap=[[0, parts], [row_stride, nrows], [1, ncols]])


def build_program(nreps=1):
    nc = bacc.Bacc("TRN2", target_bir_lowering=False, debug=False)

    dram = nc.dram_tensor
    x_f = dram("x_f", [TOK, DM], F32, kind="ExternalInput").ap()
    res_x = dram("res_x", [TL, DM], F32, kind="ExternalInput").ap()
    w_in = dram("w_in", [DM, 2 * DL], BF16, kind="ExternalInput").ap()
    f_in = dram("f_in", [2 * DL, 1], F32, kind="ExternalInput").ap()
    convw = dram("convw", [DL, DC], F32, kind="ExternalInput").ap()
    convb = dram("convb", [DL, 1], F32, kind="ExternalInput").ap()
    xpw = dram("xpw", [DL, 96], F32, kind="ExternalInput").ap()
    dtw = dram("dtw", [DTR, DL], F32, kind="ExternalInput").ap()
    dtb = dram("dtb", [DL, 1], F32, kind="ExternalInput").ap()
    negA = dram("negA", [DL, NST], F32, kind="ExternalInput").ap()
    dparam = dram("dparam", [DL, 1], F32, kind="ExternalInput").ap()
    w_out = dram("w_out", [DI, DM], BF16, kind="ExternalInput").ap()
    f_out = dram("f_out", [DM, 1], F32, kind="ExternalInput").ap()
    ident_in = dram("ident", [128, 128], BF16, kind="ExternalInput").ap()
    ident32_in = dram("ident32", [128, 128], F32, kind="ExternalInput").ap()

    out_c = dram("out_c", [DM, TL], F32, kind="ExternalOutput").ap()

    pj_in = [dram(f"pj_in{b}", [96, S], F32) for b in range(B)]
    pj_out = [dram(f"pj_out{b}", [96, S], F32) for b in range(B)]
    bc_bf = [dram(f"bc_bf{b}", [2 * NST, S], BF16) for b in range(B)]
    asc_dram = dram("asc_dram", [1, TOK], F32)
    a2a_in = [dram(f"a2a_in{b}", [NCORES, RPC, WL], BF16) for b in range(B)]
    a2a_out = [dram(f"a2a_out{b}", [NCORES, RPC, WL], BF16) for b in range(B)]
    CHK = RPC * WL             # elements per a2a chunk

    class _StopEmit(Exception):
        pass

    with tile.TileContext(nc) as tc, ExitStack() as ctx:
        pp = ctx.enter_context(tc.tile_pool(name="persist", bufs=1))
        work = ctx.enter_context(tc.tile_pool(name="work", bufs=3))
        w512 = ctx.enter_context(tc.tile_pool(name="w512", bufs=2))
        w256 = ctx.enter_context(tc.tile_pool(name="w256", bufs=3))
        scanp = ctx.enter_context(tc.tile_pool(name="scanp", bufs=2))
        tiny = ctx.enter_context(tc.tile_pool(name="tiny", bufs=4))
        psA = ctx.enter_context(tc.tile_pool(name="psA", bufs=2, space="PSUM"))
        psY = ctx.enter_context(tc.tile_pool(name="psY", bufs=3, space="PSUM"))

        # ---- constants / weights (persistent across reps) ----
        ident = pp.tile([128, 128], BF16)
        nc.sync.dma_start(out=ident, in_=ident_in)
        ident32 = pp.tile([128, 128], F32)
        nc.sync.dma_start(out=ident32, in_=ident32_in)
        w_out_t = pp.tile([128, 16, DM], BF16)
        nc.sync.dma_start(out=w_out_t, in_=w_out.rearrange("(kt p) m -> p kt m", p=128))
        f_in_t = pp.tile([128, 4], F32)
        nc.sync.dma_start(out=f_in_t, in_=f_in.rearrange("(m p) o -> p (m o)", p=128))
        convw_t = pp.tile([128, 2, DC], F32)
        nc.sync.dma_start(out=convw_t, in_=convw.rearrange("(d p) j -> p d j", p=128))
        convb_t = pp.tile([128, 2], F32)
        nc.sync.dma_start(out=convb_t, in_=convb.rearrange("(d p) o -> p (d o)", p=128))
        xpw_t = pp.tile([128, 2, 96], F32)
        nc.sync.dma_start(out=xpw_t, in_=xpw.rearrange("(kt p) m -> p kt m", p=128))
        dtw_t = pp.tile([DTR, DL], F32)
        nc.sync.dma_start(out=dtw_t, in_=dtw)
        dtb_t = pp.tile([128, 2], F32)
        nc.sync.dma_start(out=dtb_t, in_=dtb.rearrange("(d p) o -> p (d o)", p=128))
        negA_t = pp.tile([128, 2, NST], F32)
        nc.sync.dma_start(out=negA_t, in_=negA.rearrange("(d p) n -> p d n", p=128))
        dparam_t = pp.tile([128, 2], F32)
        nc.sync.dma_start(out=dparam_t, in_=dparam.rearrange("(d p) o -> p (d o)", p=128))
        f_out_t = pp.tile([128, 8], F32)
        nc.sync.dma_start(out=f_out_t, in_=f_out.rearrange("(m p) o -> p (m o)", p=128))

        eps_t = pp.tile([128, 1], F32)
        nc.vector.memset(eps_t, 1e-6)
        eps8_t = pp.tile([128, 1], F32)
        nc.vector.memset(eps8_t, 1e-8)
        one_t = pp.tile([128, 1], F32)
        nc.vector.memset(one_t, 1.0)

        def _finish_dummy():
            dz = w256.tile([128, TL], F32, tag="w256", name="dz")
            nc.vector.memset(dz, 0.0)
            for mm in range(8):
                nc.sync.dma_start(out=out_c[mm * 128:(mm + 1) * 128, :], in_=dz)

        def emit_rep():
            # w_in now; dtr_sb later reuses this slot (disjoint lifetimes)
            w_in_t = pp.tile([128, 8, 2 * DL], BF16, tag="winH", name="w_in_t")
            nc.sync.dma_start(out=w_in_t, in_=w_in.rearrange("(kt p) m -> p kt m", p=128))

            # residual: load + transpose early (PE idle window)
            res_sb = pp.tile([128, 2, DM], F32, tag="chainA", name="res_sb")
            nc.sync.dma_start(out=res_sb, in_=res_x.rearrange("(rt p) d -> p rt d", p=128))
            resT = pp.tile([128, 8, TL], F32, tag="resT_hold", name="resT")
            for m in range(8):
                psr = psA.tile([128, 512], F32, tag="ps", name="psr")
                for rt in range(2):
                    nc.tensor.transpose(psr[:, rt * 128:(rt + 1) * 128],
                                        res_sb[:, rt, m * 128:(m + 1) * 128], ident32)
                nc.scalar.copy(out=resT[:, m, :], in_=psr[:, 0:256])

            aqT = pp.tile([128, 8, TOK], BF16, tag="big4", name="aqT")
            asc_col = pp.tile([128, 16], F32, tag="asc_col", name="asc_col")
            asc_b = pp.tile([128, TOK], F32, tag="chainE", name="asc_b")
            xz = [pp.tile([128, TOK], F32, tag=t, name=f"xz{i}")
                  for i, t in enumerate(("chainA", "chainB"))]
            zsil_d = [pp.tile([128, TOK], BF16, tag=t, name=f"zsil{i}")
                      for i, t in enumerate(("chainC", "chainD"))]
            xa_d = [pp.tile([128, TOK], F32, tag=t, name=f"xa{i}")
                    for i, t in enumerate(("xa0", "xa1"))]

            def stage1(b):
                # rmsnorm + int8 quant + transpose, tokens of batch b.
                # Per-token scalar chains batched over groups of 4 row-tiles to
                # cut tiny-instruction count (HW per-instruction overhead).
                for g in range(2):
                    it0 = b * 8 + g * 4
                    mv4 = tiny.tile([128, 4, 2], F32, tag="mv4", bufs=2, name="mv4")
                    st4 = tiny.tile([128, 6, 4], F32, tag="st4", bufs=2, name="st4")
                    ss4, srt4, rstd4, amax4, qr4, qmul4 = (st4[:, i, :]
                                                           for i in range(6))
                    for j in range(4):
                        it = it0 + j
                        xt = work.tile([128, DM], F32, tag="w1k", name="xt")
                        nc.sync.dma_start(out=xt,
                                          in_=x_f[it * 128:(it + 1) * 128, :])
                        bst = tiny.tile([128, 2, 6], F32, tag="bst", name="bst")
                        for ch in range(2):
                            nc.vector.bn_stats(out=bst[:, ch, :],
                                               in_=xt[:, ch * 512:(ch + 1) * 512])
                        nc.vector.bn_aggr(out=mv4[:, j, :], in_=bst)
                        nc.vector.tensor_reduce(out=amax4[:, j:j + 1],
                                                in_=xt,
                                                axis=mybir.AxisListType.X,
                                                op=AOP.max,
                                                apply_absolute_value=True)
                    mvs = mv4.rearrange("p j c -> p c j")
                    nc.vector.tensor_tensor(out=ss4, in0=mvs[:, 0, :],
                                            in1=mvs[:, 0, :], op=AOP.mult)
                    nc.vector.tensor_tensor(out=ss4, in0=ss4, in1=mvs[:, 1, :],
                                            op=AOP.add)
                    nc.scalar.activation(out=srt4, in_=ss4, func=AF.Sqrt,
                                         scale=1.0, bias=eps_t)
                    nc.vector.reciprocal(out=rstd4, in_=srt4)
                    asc4 = asc_col[:, it0:it0 + 4]
                    nc.vector.tensor_tensor(out=asc4, in0=amax4, in1=rstd4,
                                            op=AOP.mult)
                    nc.vector.tensor_scalar(out=asc4, in0=asc4, scalar1=1e-8,
                                            scalar2=None, op0=AOP.add)
                    nc.vector.reciprocal(out=qr4, in_=asc4)
                    nc.vector.tensor_tensor(out=qmul4, in0=qr4, in1=rstd4,
                                            op=AOP.mult)
                    nc.vector.tensor_scalar(out=qmul4, in0=qmul4, scalar1=127.0,
                                            scalar2=None, op0=AOP.mult)
                    for j in range(4):
                        it = it0 + j
                        xt2 = work.tile([128, DM], F32, tag="w1k", name="xt2")
                        nc.scalar.dma_start(out=xt2,
                                            in_=x_f[it * 128:(it + 1) * 128, :])
                        aq32 = work.tile([128, DM], F32, tag="w1k", name="aq32")
                        nc.scalar.activation(out=aq32, in_=xt2, func=AF.Copy,
                                             scale=qmul4[:, j:j + 1])
                        aqb = work.tile([128, DM], BF16, tag="aqb", bufs=1, name="aqb")
                        (nc.gpsimd if STG1_AQB == "pool" else nc.vector).tensor_scalar(
                            out=aqb, in0=aq32, scalar1=MAGIC,
                            scalar2=MAGIC, op0=AOP.add, op1=AOP.subtract)
                        for half in range(2):
                            pst = psA.tile([128, 512], BF16, tag="ps", name="pst")
                            for jj in range(4):
                                k = half * 4 + jj
                                nc.tensor.transpose(pst[:, jj * 128:(jj + 1) * 128],
                                                    aqb[:, k * 128:(k + 1) * 128],
                                                    ident)
                            if STG1_CP == "act":
                                nc.scalar.copy(
                                    out=aqT[:, half * 4:(half + 1) * 4,
                                            it * 128:(it + 1) * 128],
                                    in_=pst.rearrange("p (j f) -> p j f", j=4))
                            else:
                                nc.vector.tensor_copy(
                                    out=aqT[:, half * 4:(half + 1) * 4,
                                            it * 128:(it + 1) * 128],
                                    in_=pst.rearrange("p (j f) -> p j f", j=4))
                nc.sync.dma_start(
                    out=bass.AP(tensor=asc_dram.ap().tensor, offset=b * S,
                                ap=[[1, 128], [128, 8]]),
                    in_=asc_col[:, b * 8:b * 8 + 8])
                nc.gpsimd.dma_start(
                    out=asc_b[:, b * S:(b + 1) * S],
                    in_=bass.AP(tensor=asc_dram.ap().tensor, offset=b * S,
                                ap=[[0, 128], [1, S]]))

            def in_proj(b):
                for m in range(4):
                    for tch in range(2 * b, 2 * b + 2):
                        ps = psA.tile([128, 512], F32, tag="ps", name="ps")
                        for k in range(8):
                            nc.tensor.matmul(ps, w_in_t[:, k, m * 128:(m + 1) * 128],
                                             aqT[:, k, tch * 512:(tch + 1) * 512],
                                             start=(k == 0), stop=(k == 7))
                        t1 = w512.tile([128, 512], F32, tag="w512", name="t1")
                        if STG1_T1 == "act":
                            nc.scalar.activation(out=t1, in_=ps, func=AF.Copy,
                                                 scale=f_in_t[:, m:m + 1])
                        else:
                            nc.vector.tensor_scalar(out=t1, in0=ps,
                                                    scalar1=f_in_t[:, m:m + 1],
                                                    scalar2=None, op0=AOP.mult)
                        if m < 2:
                            nc.vector.tensor_tensor(
                                out=xz[m][:, tch * 512:(tch + 1) * 512], in0=t1,
                                in1=asc_b[:, tch * 512:(tch + 1) * 512],
                                op=AOP.mult)
                        else:
                            t2 = w512.tile([128, 512], F32, tag="w512", name="t2")
                            nc.vector.tensor_tensor(
                                out=t2, in0=t1,
                                in1=asc_b[:, tch * 512:(tch + 1) * 512],
                                op=AOP.mult)
                            nc.scalar.activation(
                                out=zsil_d[m - 2][:, tch * 512:(tch + 1) * 512],
                                in_=t2, func=AF.Silu)

            # ---- stages 1-2 upfront (keeps scan windows free of front-end
            # work on shared engines), then conv+x_proj+AllReduce per b ----
            for b in range(B):
                stage1(b)
                in_proj(b)
            for b in range(B):
                if KSTOP == "s2":
                    continue
                for d in range(2):
                    pad = work.tile([128, S + 3], F32, tag="w1k", name="pad")
                    nc.vector.memset(pad[:, 0:3], 0.0)
                    nc.scalar.copy(out=pad[:, 3:S + 3], in_=xz[d][:, b * S:(b + 1) * S])
                    c0 = work.tile([128, S], F32, tag="w1k", name="c0")
                    nc.vector.tensor_scalar(out=c0, in0=pad[:, 0:S],
                                            scalar1=convw_t[:, d, 0:1], scalar2=None,
                                            op0=AOP.mult)
                    c1 = work.tile([128, S], F32, tag="w1k", name="c1")
                    nc.vector.scalar_tensor_tensor(out=c1, in0=pad[:, 1:S + 1],
                                                   scalar=convw_t[:, d, 1:2], in1=c0,
                                                   op0=AOP.mult, op1=AOP.add)
                    c2 = work.tile([128, S], F32, tag="w1k", name="c2")
                    nc.vector.scalar_tensor_tensor(out=c2, in0=pad[:, 2:S + 2],
                                                   scalar=convw_t[:, d, 2:3], in1=c1,
                                                   op0=AOP.mult, op1=AOP.add)
                    c3 = work.tile([128, S], F32, tag="w1k", name="c3")
                    nc.vector.scalar_tensor_tensor(out=c3, in0=pad[:, 3:S + 3],
                                                   scalar=convw_t[:, d, 3:4], in1=c2,
                                                   op0=AOP.mult, op1=AOP.add)
                    nc.scalar.activation(out=xa_d[d][:, b * S:(b + 1) * S], in_=c3,
                                         func=AF.Silu, bias=convb_t[:, d:d + 1])
                for tch in range(2):
                    ps96 = psA.tile([96, 512], F32, tag="ps", name="ps96")
                    c0_ = b * S + tch * 512
                    for k in range(2):
                        nc.tensor.matmul(ps96, xpw_t[:, k, :],
                                         xa_d[k][:, c0_:c0_ + 512],
                                         start=(k == 0), stop=(k == 1))
                    pjc = w512.tile([96, 512], F32, tag="w512", name="pjc")
                    nc.scalar.copy(out=pjc, in_=ps96)
                    nc.sync.dma_start(out=pj_in[b][:, tch * 512:(tch + 1) * 512],
                                      in_=pjc)
                if COLL == "real":
                    nc.gpsimd.collective_compute(
                        "AllReduce", AOP.add, ins=[pj_in[b][:]], outs=[pj_out[b][:]],
                        replica_groups=[list(range(NCORES))])
                else:
                    nc.sync.dma_start(out=pj_out[b][:], in_=pj_in[b][:])

            if KSTOP == "s2":
                raise _StopEmit()

            # ---- per b: dt_proj + scan + ship + AllToAll(b) ----
            dtall = pp.tile([128, 2, 2, S], BF16, tag="chainA", name="dtall")  # [b][m]
            dtxb = pp.tile([128, 2, TOK], BF16, tag="chainE", name="dtxb")
            e_full = pp.tile([128, 2, TOK], F32, tag="big4", name="e_full")
            gp_i = 0

            def tt_engine():
                nonlocal gp_i
                gp_i += 1
                return nc.gpsimd if (gp_i % GP_FRAC_DEN) < GP_FRAC_NUM else nc.vector

            NGRP = 1   # states per batched broadcast DMA
            y_ps_all = {}
            y2s_all = {}
            for b in range(B):
                dtr_sb = pp.tile([64, S], F32, tag="winH", name=f"dtr_sb{b}")
                nc.sync.dma_start(out=dtr_sb, in_=pj_out[b][0:64, :])
                bcbf_sb = work.tile([2 * NST, S], BF16, tag="w1k", name=f"bcbf_sb{b}")
                nc.gpsimd.dma_start(out=bcbf_sb, in_=pj_out[b][64:96, :])
                nc.sync.dma_start(out=bc_bf[b][:], in_=bcbf_sb)

                # dt_proj: batch the Exps then the Lns (ACT table locality)
                for m in range(2):
                    for tch in range(2):
                        psd = psA.tile([128, 512], F32, tag="ps", name="psd")
                        nc.tensor.matmul(psd, dtw_t[:, m * 128:(m + 1) * 128],
                                         dtr_sb[:, tch * 512:(tch + 1) * 512],
                                         start=True, stop=True)
                        nc.scalar.activation(
                            out=e_full[:, m, b * S + tch * 512:b * S + (tch + 1) * 512],
                            in_=psd, func=AF.Exp, scale=1.0, bias=dtb_t[:, m:m + 1])
                for m in range(2):
                    for tch in range(2):
                        sl = slice(b * S + tch * 512, b * S + (tch + 1) * 512)
                        nc.scalar.activation(out=dtall[:, b, m, tch * 512:(tch + 1) * 512],
                                             in_=e_full[:, m, sl],
                                             func=AF.Ln, bias=one_t)
                    (nc.gpsimd if DTXB_POOL else nc.vector).tensor_tensor(
                        out=dtxb[:, m, b * S:(b + 1) * S],
                        in0=dtall[:, b, m, :],
                        in1=xa_d[m][:, b * S:(b + 1) * S],
                        op=AOP.mult)

                # ---- scan(b) ----
                y_ps = y_ps_all[b] = {}
                for d in range(2):
                    y_ps[d] = psY.tile([128, S], F32, tag="yps", name=f"yps{b}{d}")
                for g in range(NST // NGRP):
                    n0 = g * NGRP
                    bgrp = scanp.tile([128, NGRP, S], BF16, tag="bgrp", bufs=2,
                                      name="bgrp")
                    nc.sync.dma_start(
                        out=bgrp, in_=_bcast_rows(bc_bf[b], n0, NGRP, 0, S, S))
                    cgrp = scanp.tile([128, NGRP, S], BF16, tag="cgrp", bufs=2,
                                      name="cgrp")
                    nc.sync.dma_start(
                        out=cgrp, in_=_bcast_rows(bc_bf[b], NST + n0, NGRP, 0, S, S))
                    for j in range(NGRP):
                        n = n0 + j
                        bj = bgrp[:, j, :]
                        bj2 = bass.AP(tensor=bj.tensor, offset=bj.offset,
                                      ap=[bj.ap[0], [0, 2], bj.ap[1]])
                        cj = cgrp[:, j, :]
                        cj2 = bass.AP(tensor=cj.tensor, offset=cj.offset,
                                      ap=[cj.ap[0], [0, 2], cj.ap[1]])
                        u = scanp.tile([128, 2, S], BF16 if U_BF16 else F32, tag="u",
                                       bufs=2, name="u")
                        tt_engine().tensor_tensor(
                            out=u, in0=dtxb[:, :, b * S:(b + 1) * S],
                            in1=bj2, op=AOP.mult)
                        h = scanp.tile([128, 2, S], BF16 if H_BF16 else F32, tag="h",
                                       bufs=2, name="h")
                        for d in range(2):
                            dA = scanp.tile([128, S], BF16 if DA_BF16 else F32,
                                            tag="dA", bufs=2, name="dA")
                            nc.scalar.activation(out=dA, in_=dtall[:, b, d, :],
                                                 func=AF.Exp,
                                                 scale=negA_t[:, d, n:n + 1])
                            nc.vector.tensor_tensor_scan(h[:, d, :], dA, u[:, d, :],
                                                         0.0, AOP.mult, AOP.add)
                        w = scanp.tile([128, 2, S], BF16, tag="w", bufs=2, name="w")
                        tt_engine().tensor_tensor(out=w, in0=h, in1=cj2, op=AOP.mult)
                        for d in range(2):
                            for hf in range(2):
                                nc.tensor.matmul(y_ps[d][:, hf * 512:(hf + 1) * 512],
                                                 ident,
                                                 w[:, d, hf * 512:(hf + 1) * 512],
                                                 start=(n == 0), stop=(n == NST - 1))

                # y2 epilogue right after scan(b): frees the psY slots so
                # scan(b+1)'s PSUM accumulation isn't stalled on rotation
                y2s_all[b] = []
                for d in range(2):
                    y2 = work.tile([128, S], BF16, tag=f"y2_{b}_{d}", bufs=1,
                                   name="y2")
                    nc.vector.scalar_tensor_tensor(
                        out=y2, in0=y_ps[d], scalar=dparam_t[:, d:d + 1],
                        in1=zsil_d[d][:, b * S:(b + 1) * S],
                        op0=AOP.mult, op1=AOP.mult)
                    y2s_all[b].append(y2)

            # ---- ship(b) + AllToAll(b), after both scans so the collective's
            # queue-held waits never block scan instructions ----
            for b in range(B):
                y2s = y2s_all[b]
                prs = []
                for d in range(2):
                    y2 = y2s[d]
                    if SHIP_MAX == "piggyback":
                        pr = work.tile([128, S], F32, tag=f"pr_{d}", bufs=1, name="pr")
                        nc.gpsimd.partition_all_reduce(pr, y2, 128,
                                                       bass_isa.ReduceOp.absmax)
                        prs.append(pr)
                    nc.sync.dma_start(
                        out=bass.AP(tensor=a2a_in[b].ap().tensor,
                                    offset=(d * 128) * WL,
                                    ap=[[WL, 128], [CHK, 8], [1, WL]]),
                        in_=y2.rearrange("p (j t) -> p j t", j=8))
                if SHIP_MAX == "piggyback":
                    mrow = work.tile([128, S], BF16, tag="y2_0", bufs=1, name="mrow")
                    nc.vector.tensor_tensor(out=mrow, in0=prs[0], in1=prs[1],
                                            op=AOP.max)
                    nc.sync.dma_start(
                        out=bass.AP(tensor=a2a_in[b].ap().tensor,
                                    offset=(2 * 128) * WL,
                                    ap=[[0, 1], [CHK, 8], [1, WL]]),
                        in_=mrow[0:1, :])
                if COLL == "real":
                    nc.gpsimd.collective_compute(
                        "AllToAll", AOP.bypass, ins=[a2a_in[b][:]],
                        outs=[a2a_out[b][:]],
                        replica_groups=[list(range(NCORES))])
                else:
                    nc.sync.dma_start(out=a2a_out[b].ap().rearrange("c r t -> (c r) t"),
                                      in_=a2a_in[b].ap().rearrange("c r t -> (c r) t"))

            if KSTOP == "scan":
                raise _StopEmit()

            # ---- stage 8 per wave: out quant + out_proj + residual ----
            for b in range(B):
                y2f = pp.tile([128, 8, 2, WL], BF16, tag="big4", name=f"y2f{b}")
                for k2 in range(2):
                    nc.sync.dma_start(
                        out=y2f[:, :, k2, :],
                        in_=bass.AP(tensor=a2a_out[b].ap().tensor,
                                    offset=k2 * 128 * WL,
                                    ap=[[WL, 128], [CHK, 8], [1, WL]]))
                sm8 = tiny.tile([128, 3, WL], F32, tag="sm8", bufs=1, name="sm8")
                asco, recq, m1v = (sm8[:, i, :] for i in range(3))
                if SHIP_MAX == "piggyback":
                    mx8 = tiny.tile([8, 2, WL], BF16, tag="mx8", bufs=1, name="mx8")
                    nc.gpsimd.dma_start(
                        out=mx8[:, 0, :],
                        in_=bass.AP(tensor=a2a_out[b].ap().tensor,
                                    offset=2 * 128 * WL,
                                    ap=[[CHK, 8], [1, WL]]))
                    nc.gpsimd.partition_all_reduce(mx8[:, 1, :], mx8[:, 0, :], 8,
                                                   bass_isa.ReduceOp.max)
                    mxb = tiny.tile([128, WL], BF16, tag="mxb", bufs=1, name="mxb")
                    nc.gpsimd.partition_broadcast(mxb, mx8[0:1, 1, :])
                    nc.vector.tensor_scalar(out=asco, in0=mxb, scalar1=1e-8,
                                            scalar2=None, op0=AOP.add)
                else:
                    nc.vector.tensor_reduce(
                        out=m1v, in_=y2f.rearrange("p c k t -> p t c k"),
                        axis=mybir.AxisListType.XY, op=AOP.max,
                        apply_absolute_value=True)
                    nc.gpsimd.partition_all_reduce(asco, m1v, 128,
                                                   bass_isa.ReduceOp.absmax)
                    nc.vector.tensor_scalar(out=asco, in0=asco, scalar1=1e-8,
                                            scalar2=None, op0=AOP.add)
                nc.vector.reciprocal(out=recq, in_=asco)
                nc.vector.tensor_scalar(out=recq, in0=recq, scalar1=127.0, scalar2=None,
                                        op0=AOP.mult)
                aqo = pp.tile([128, 16, WL], BF16, tag="xa1", name=f"aqo{b}")
                for kt in range(16):
                    c8, k2 = kt // 2, kt % 2
                    q1 = w256.tile([128, WL], F32, tag="w256", name="q1")
                    nc.vector.tensor_tensor(out=q1, in0=y2f[:, c8, k2, :], in1=recq,
                                            op=AOP.mult)
                    nc.vector.tensor_scalar(out=aqo[:, kt, :], in0=q1, scalar1=MAGIC,
                                            scalar2=MAGIC, op0=AOP.add,
                                            op1=AOP.subtract)
                for m in range(8):
                    pso = psA.tile([128, WL], F32, tag="ps", name="pso")
                    for k in range(16):
                        c8, k2 = k // 2, k % 2
                        nc.tensor.matmul(pso,
                                         w_out_t[:, c8 * 2 + k2, m * 128:(m + 1) * 128],
                                         aqo[:, k, :], start=(k == 0), stop=(k == 15))
                    fin = w256.tile([128, WL], F32, tag="w256", name="fin")
                    nc.vector.scalar_tensor_tensor(out=fin, in0=pso,
                                                   scalar=f_out_t[:, m:m + 1], in1=asco,
                                                   op0=AOP.mult, op1=AOP.mult)
                    outm = w256.tile([128, WL], F32, tag="w256", name="outm")
                    nc.vector.tensor_tensor(out=outm, in0=fin,
                                            in1=resT[:, m, b * WL:(b + 1) * WL],
                                            op=AOP.add)
                    nc.sync.dma_start(
                        out=out_c[m * 128:(m + 1) * 128, b * WL:(b + 1) * WL],
                        in_=outm)

        stopped = False
        for _rep in range(nreps):
            try:
                emit_rep()
            except _StopEmit:
                stopped = True
        if stopped:
            _finish_dummy()

    nc.compile()
    return nc


def _host_prep(inputs):
    x = np.asarray(inputs["x"], np.float32)
    norm_w = np.asarray(inputs["norm_w"], np.float32)
    in_w = np.asarray(inputs["in_w"], np.float32)
    in_alpha = np.asarray(inputs["in_alpha"], np.float32)
    conv_w = np.asarray(inputs["conv_w"], np.float32)
    conv_b = np.asarray(inputs["conv_b"], np.float32)
    xproj_w = np.asarray(inputs["xproj_w"], np.float32)
    dt_w = np.asarray(inputs["dt_w"], np.float32)
    dt_b = np.asarray(inputs["dt_b"], np.float32)
    log_A = np.asarray(inputs["log_A"], np.float32)
    d_param = np.asarray(inputs["d_param"], np.float32)
    out_w = np.asarray(inputs["out_w"], np.float32)
    out_alpha = np.asarray(inputs["out_alpha"], np.float32)

    assert np.allclose(norm_w, 1.0), "general norm_w path not wired on device"

    xf = np.ascontiguousarray(x.reshape(TOK, DM))

    ws_in = np.abs(in_w).mean(axis=1, keepdims=True) + 1e-8
    wq_in = np.clip(np.round(in_w / ws_in), -1, 1).astype(np.float32)
    fin_full = (ws_in[:, 0] * in_alpha / 127.0).astype(np.float32)

    ws_out = np.abs(out_w).mean(axis=1, keepdims=True) + 1e-8
    wq_out = np.clip(np.round(out_w / ws_out), -1, 1).astype(np.float32)
    fout_full = (ws_out[:, 0] * out_alpha / 127.0).astype(np.float32)

    A = np.exp(log_A).astype(np.float32)
    ident = np.eye(128, dtype=ml_dtypes.bfloat16)
    ident32 = np.eye(128, dtype=np.float32)
    w_out_bf = np.ascontiguousarray(wq_out.T).astype(ml_dtypes.bfloat16)

    in_maps = []
    for c in range(NCORES):
        sl = slice(c * DL, (c + 1) * DL)
        rows = np.r_[c * DL:(c + 1) * DL, DI + c * DL:DI + (c + 1) * DL]
        # residual rows: wave0 = batch-0 tokens [c*128:(c+1)*128],
        #                wave1 = batch-1 tokens [S + c*128 : S + (c+1)*128]
        res_rows = np.r_[c * WL:(c + 1) * WL, S + c * WL:S + (c + 1) * WL]
        in_maps.append(dict(
            x_f=xf,
            res_x=np.ascontiguousarray(xf[res_rows, :]),
            w_in=np.ascontiguousarray(wq_in[rows].T).astype(ml_dtypes.bfloat16),
            f_in=np.ascontiguousarray(fin_full[rows][:, None]),
            convw=np.ascontiguousarray(conv_w[sl, 0, :]),
            convb=np.ascontiguousarray(conv_b[sl][:, None]),
            xpw=np.ascontiguousarray(xproj_w[:, sl].T),
            dtw=np.ascontiguousarray(dt_w[sl, :].T),
            dtb=np.ascontiguousarray(dt_b[sl][:, None]),
            negA=np.ascontiguousarray(-A[sl, :]),
            dparam=np.ascontiguousarray(d_param[sl][:, None]),
            w_out=w_out_bf,
            f_out=np.ascontiguousarray(fout_full[:, None]),
            ident=ident,
            ident32=ident32,
        ))
    return in_maps


def kernel(**inputs):
    if "nc" not in _CACHE:
        _CACHE["nc"] = build_program()
    nc = _CACHE["nc"]
    in_maps = _host_prep(inputs)
    res = run_bass_kernel_spmd(nc, in_maps, list(range(NCORES)))
    _CACHE["last_results"] = res
    out = np.empty((TOK, DM), np.float32)
    for c in range(NCORES):
        oc = res.results[c]["out_c"]
        out[c * WL:(c + 1) * WL, :] = oc[:, 0:WL].T
        out[S + c * WL:S + (c + 1) * WL, :] = oc[:, WL:2 * WL].T
    return out.reshape(B, S, DM)
